# revision 6
# baseline (speedup 1.0000x reference)
"""Trainium2 Bass kernel for CustomMHA (B=2, T=2048, D=768, H=12, Dh=64).

Sharding: 8 cores = 2 batches x 4 head-groups (3 heads each).
Each core computes, for its (b, heads) shard:
  - Q^T, K^T (transposed layouts) and V (natural) projections
  - scores in BOTH layouts on PE (natural for softmax + A output,
    transposed for the A@V contraction)
  - no-max softmax: exp on ACT with fused row-sum (accum_out),
    A = gate/rowsum * E scaled on DVE, streamed to DRAM
  - z^T = V^T E^T accumulated in PSUM, scaled by gate/rowsum, then
    out_partial = z @ Wo_rows on PE.
Host: gathers A shards, sums the 4 per-batch out partials, adds
bo and the (exact) bv correction  sum_h gate[b,h] * (bv_h @ Wo_h).
"""

import sys

if "/opt/trn_rl_repo" not in sys.path:
    sys.path.insert(0, "/opt/trn_rl_repo")

import math

import numpy as np

import concourse.bass as bass
import concourse.tile as tile
import concourse.mybir as mybir
from concourse import bacc
from concourse.bass_utils import run_bass_kernel_spmd
from concourse.masks import make_identity

F32 = mybir.dt.float32
AF = mybir.ActivationFunctionType

D_MODEL = 768
N_HEADS = 12
D_HEAD = 64
B, T = 2, 2048
HPC = 3          # heads per core
NB = T // 128    # 16 blocks of 128
KC = D_MODEL // 128  # 6 contraction chunks


def build_program():
    nc = bacc.Bacc("TRN2", target_bir_lowering=False, debug=False, num_devices=8)

    xb = nc.dram_tensor("x", [T, D_MODEL], F32, kind="ExternalInput")
    wq = nc.dram_tensor("wq", [D_MODEL, HPC * D_HEAD], F32, kind="ExternalInput")
    wk = nc.dram_tensor("wk", [D_MODEL, HPC * D_HEAD], F32, kind="ExternalInput")
    wv = nc.dram_tensor("wv", [D_MODEL, HPC * D_HEAD], F32, kind="ExternalInput")
    wo = nc.dram_tensor("wo", [HPC * D_HEAD, D_MODEL], F32, kind="ExternalInput")
    bq = nc.dram_tensor("bq", [HPC * D_HEAD, 1], F32, kind="ExternalInput")
    bk = nc.dram_tensor("bk", [HPC * D_HEAD, 1], F32, kind="ExternalInput")
    gmat = nc.dram_tensor("gmat", [128, HPC], F32, kind="ExternalInput")

    A_out = nc.dram_tensor("A", [HPC, T, T], F32, kind="ExternalOutput")
    o_out = nc.dram_tensor("out", [T, D_MODEL], F32, kind="ExternalOutput")

    c_dram = nc.dram_tensor("cscratch", [1, NB * 128], F32)

    from contextlib import ExitStack
    stack = ExitStack()
    with tile.TileContext(nc) as tc:
        consts = stack.enter_context(tc.tile_pool(name="consts", bufs=1))
        ident = consts.tile([128, 128], F32)
        make_identity(nc, ident[:])
        wq_t = consts.tile([128, KC, HPC * D_HEAD], F32)
        wk_t = consts.tile([128, KC, HPC * D_HEAD], F32)
        wv_t = consts.tile([128, KC, HPC * D_HEAD], F32)
        nc.sync.dma_start(out=wq_t[:], in_=wq.rearrange("(k p) m -> p k m", p=128))
        nc.sync.dma_start(out=wk_t[:], in_=wk.rearrange("(k p) m -> p k m", p=128))
        nc.sync.dma_start(out=wv_t[:], in_=wv.rearrange("(k p) m -> p k m", p=128))
        wo_t = consts.tile([128, 2, D_MODEL], F32)
        nc.sync.dma_start(out=wo_t[:, 0, :], in_=wo[0:128, :])
        nc.sync.dma_start(out=wo_t[0:64, 1, :], in_=wo[128:192, :])
        bq_sb = consts.tile([128, 2], F32)
        bk_sb = consts.tile([128, 2], F32)
        nc.sync.dma_start(out=bq_sb[:, 0:1], in_=bq[0:128, :])
        nc.sync.dma_start(out=bq_sb[0:64, 1:2], in_=bq[128:192, :])
        nc.sync.dma_start(out=bk_sb[:, 0:1], in_=bk[0:128, :])
        nc.sync.dma_start(out=bk_sb[0:64, 1:2], in_=bk[128:192, :])
        gm_sb = consts.tile([128, HPC], F32)
        nc.sync.dma_start(out=gm_sb[:], in_=gmat[:])
        ones1 = consts.tile([1, 64], F32)
        nc.gpsimd.memset(ones1[:], 1.0)

        qkv = stack.enter_context(tc.tile_pool(name="qkv", bufs=1))
        qT01 = qkv.tile([128, T], F32)
        qT2 = qkv.tile([64, T], F32)
        kT01 = qkv.tile([128, T], F32)
        kT2 = qkv.tile([64, T], F32)
        V = qkv.tile([128, NB, HPC * D_HEAD], F32)

        # ---- phase 1: load x, build x^T via PE transposes; also V ----
        with tc.tile_pool(name="xpools", bufs=1) as xpool, \
             tc.tile_pool(name="xTpool", bufs=1) as xTpool:
            x_all = xpool.tile([128, NB, D_MODEL], F32)
            nc.sync.dma_start(
                out=x_all[:], in_=xb.rearrange("(i p) d -> p i d", p=128)
            )
            xT = xTpool.tile([128, KC, T], F32)
            with tc.tile_pool(name="psum_tr", bufs=4, space="PSUM") as ptr, \
                 tc.tile_pool(name="psum_v", bufs=2, space="PSUM") as pv:
                for i in range(NB):
                    for k in range(KC):
                        pt = ptr.tile([128, 128], F32)
                        nc.tensor.transpose(
                            pt[:], x_all[:, i, k * 128:(k + 1) * 128], ident[:]
                        )
                        nc.vector.tensor_copy(
                            xT[:, k, i * 128:(i + 1) * 128], pt[:]
                        )
                # V natural: [t, dh'] via lhsT = xT chunk, rhs = wv chunk
                for i in range(NB):
                    pvt = pv.tile([128, HPC * D_HEAD], F32)
                    for k in range(KC):
                        nc.tensor.matmul(
                            pvt[:],
                            xT[:, k, i * 128:(i + 1) * 128],
                            wv_t[:, k, :],
                            start=(k == 0),
                            stop=(k == KC - 1),
                        )
                    nc.vector.tensor_copy(V[:, i, :], pvt[:])

            # ---- phase 2: Q^T / K^T (stacked head pairs) ----
            with tc.tile_pool(name="psum_qkv", bufs=2, space="PSUM") as pq:
                for (dst, wt, bsb, lo, hi, col) in (
                    (qT01, wq_t, bq_sb, 0, 128, 0),
                    (qT2, wq_t, bq_sb, 128, 192, 1),
                    (kT01, wk_t, bk_sb, 0, 128, 0),
                    (kT2, wk_t, bk_sb, 128, 192, 1),
                ):
                    m = hi - lo
                    pt = pq.tile([128, T], F32, tag="pqkv")
                    for k in range(KC):
                        for n in range(4):
                            nc.tensor.matmul(
                                pt[0:m, n * 512:(n + 1) * 512],
                                wt[:, k, lo:hi],
                                xT[:, k, n * 512:(n + 1) * 512],
                                start=(k == 0),
                                stop=(k == KC - 1),
                            )
                    nc.vector.tensor_scalar(
                        out=dst[:], in0=pt[0:m, :],
                        scalar1=bsb[0:m, col:col + 1], scalar2=None,
                        op0=mybir.AluOpType.add,
                    )

        # ---- per-head attention ----
        epool = stack.enter_context(tc.tile_pool(name="epool", bufs=3))
        etpool = stack.enter_context(tc.tile_pool(name="etpool", bufs=3))
        cbpool = stack.enter_context(tc.tile_pool(name="cbpool", bufs=2))
        zpool = stack.enter_context(tc.tile_pool(name="zpool", bufs=1))
        small = stack.enter_context(tc.tile_pool(name="small", bufs=2))
        zTs_all = zpool.tile([128, T], F32)
        zTs2 = zpool.tile([64, T], F32)

        for h in range(HPC):
            if h < 2:
                qT_h = qT01[h * 64:(h + 1) * 64, :]
                kT_h = kT01[h * 64:(h + 1) * 64, :]
            else:
                qT_h = qT2[0:64, :]
                kT_h = kT2[0:64, :]

            r_all = small.tile([128, NB], F32, tag="r_all")
            c_all = small.tile([128, NB], F32, tag="c_all")
            tmp_r = small.tile([128, NB], F32, tag="tmp_r")

            # natural phase: softmax + A output
            with tc.tile_pool(name="psum_S", bufs=2, space="PSUM") as ps:
                for i in range(NB):
                    pt = ps.tile([128, T], F32, tag="S")
                    for n in range(4):
                        nc.tensor.matmul(
                            pt[:, n * 512:(n + 1) * 512],
                            qT_h[:, i * 128:(i + 1) * 128],
                            kT_h[:, n * 512:(n + 1) * 512],
                            start=True,
                            stop=True,
                        )
                    E = epool.tile([128, T], F32, tag="E")
                    nc.scalar.activation(
                        out=E[:], in_=pt[:], func=AF.Exp,
                        accum_out=r_all[:, i:i + 1],
                    )
                    nc.vector.reciprocal(
                        out=tmp_r[:, i:i + 1], in_=r_all[:, i:i + 1]
                    )
                    nc.vector.tensor_tensor(
                        out=c_all[:, i:i + 1],
                        in0=tmp_r[:, i:i + 1],
                        in1=gm_sb[:, h:h + 1],
                        op=mybir.AluOpType.mult,
                    )
                    nc.vector.tensor_scalar(
                        out=E[:], in0=E[:],
                        scalar1=c_all[:, i:i + 1], scalar2=None,
                        op0=mybir.AluOpType.mult,
                    )
                    nc.sync.dma_start(
                        out=A_out[h, i * 128:(i + 1) * 128, :], in_=E[:]
                    )

            # build C_bcast [64, T]: c as a free-dim row, replicated
            C_b = cbpool.tile([64, T], F32, tag="cb")
            with tc.tile_pool(name="psum_cb", bufs=1, space="PSUM") as pcb, \
                 tc.tile_pool(name="psum_ct", bufs=1, space="PSUM") as pct:
                ptc = pct.tile([NB, 128], F32)
                nc.tensor.transpose(ptc[:], c_all[:], ident[:])
                cT_sb = small.tile([NB, 128], F32, tag="cT")
                nc.vector.tensor_copy(cT_sb[:], ptc[:])
                nc.sync.dma_start(
                    out=c_dram.rearrange("a (i p) -> (a i) p", i=NB),
                    in_=cT_sb[:],
                )
                c_free = small.tile([1, T], F32, tag="cfree")
                nc.sync.dma_start(out=c_free[:], in_=c_dram[:])
                pcbt = pcb.tile([64, T], F32)
                for n in range(4):
                    nc.tensor.matmul(
                        pcbt[:, n * 512:(n + 1) * 512],
                        ones1[:],
                        c_free[:, n * 512:(n + 1) * 512],
                        start=True,
                        stop=True,
                    )
                nc.vector.tensor_copy(C_b[:], pcbt[:])

            # transposed phase: S^T -> exp -> z^T accumulation
            with tc.tile_pool(name="psum_ST", bufs=2, space="PSUM") as pst, \
                 tc.tile_pool(name="psum_zT", bufs=1, space="PSUM") as pzt:
                zT = pzt.tile([64, T], F32)
                for j in range(NB):
                    for half in range(2):
                        pt = pst.tile([128, 1024], F32, tag="ST")
                        for k2 in range(2):
                            nc.tensor.matmul(
                                pt[:, k2 * 512:(k2 + 1) * 512],
                                kT_h[:, j * 128:(j + 1) * 128],
                                qT_h[:, half * 1024 + k2 * 512:
                                     half * 1024 + (k2 + 1) * 512],
                                start=True,
                                stop=True,
                            )
                        ET = etpool.tile([128, 1024], F32, tag="ET")
                        nc.scalar.activation(out=ET[:], in_=pt[:], func=AF.Exp)
                        for k2 in range(2):
                            nc.tensor.matmul(
                                zT[:, half * 1024 + k2 * 512:
                                   half * 1024 + (k2 + 1) * 512],
                                V[:, j, h * 64:(h + 1) * 64],
                                ET[:, k2 * 512:(k2 + 1) * 512],
                                start=(j == 0),
                                stop=(j == NB - 1),
                                skip_group_check=True,
                            )
                # scale columns by c (gate/rowsum) and park in SBUF
                if h < 2:
                    zdst = zTs_all[h * 64:(h + 1) * 64, :]
                else:
                    zdst = zTs2[0:64, :]
                nc.vector.tensor_tensor(
                    out=zdst, in0=zT[:], in1=C_b[:], op=mybir.AluOpType.mult
                )

        # ---- output projection ----
        with tc.tile_pool(name="psum_out", bufs=2, space="PSUM") as po, \
             tc.tile_pool(name="obuf", bufs=3) as obuf:
            for i2 in range(NB // 2):
                ot = obuf.tile([128, 2, D_MODEL], F32, tag="ot")
                for ii in range(2):
                    i = i2 * 2 + ii
                    pt = po.tile([128, D_MODEL], F32, tag="po")
                    for n, (n0, n1) in enumerate(((0, 512), (512, 768))):
                        nc.tensor.matmul(
                            pt[:, n0:n1],
                            zTs_all[:, i * 128:(i + 1) * 128],
                            wo_t[:, 0, n0:n1],
                            start=True,
                            stop=False,
                        )
                        nc.tensor.matmul(
                            pt[:, n0:n1],
                            zTs2[:, i * 128:(i + 1) * 128],
                            wo_t[0:64, 1, n0:n1],
                            start=False,
                            stop=True,
                        )
                    nc.vector.tensor_copy(ot[:, ii, :], pt[:])
                nc.sync.dma_start(
                    out=o_out[i2 * 256:(i2 + 1) * 256, :].rearrange(
                        "(g p) d -> p g d", p=128
                    ),
                    in_=ot[:],
                )

        stack.close()

    nc.compile()
    return nc


_NC = None


def _get_nc():
    global _NC
    if _NC is None:
        _NC = build_program()
    return _NC


def make_in_maps(x, gates, Wq, bq, Wk, bk, Wv, bv, Wo, bo):
    x = np.asarray(x, np.float32)
    gates = np.asarray(gates, np.float32)
    Wq, bq = np.asarray(Wq, np.float32), np.asarray(bq, np.float32)
    Wk, bk = np.asarray(Wk, np.float32), np.asarray(bk, np.float32)
    Wv = np.asarray(Wv, np.float32)
    Wo = np.asarray(Wo, np.float32)
    in_maps = []
    for c in range(8):
        b, g = c // 4, c % 4
        sl = slice(g * HPC * D_HEAD, (g + 1) * HPC * D_HEAD)
        gm = np.repeat(gates[b, g * HPC:(g + 1) * HPC][None, :], 128, axis=0)
        in_maps.append({
            "x": np.ascontiguousarray(x[b]),
            "wq": np.ascontiguousarray(Wq[:, sl]) / 8.0,
            "wk": np.ascontiguousarray(Wk[:, sl]),
            "wv": np.ascontiguousarray(Wv[:, sl]),
            "wo": np.ascontiguousarray(Wo[sl, :]),
            "bq": np.ascontiguousarray(bq[sl, None]) / 8.0,
            "bk": np.ascontiguousarray(bk[sl, None]),
            "gmat": np.ascontiguousarray(gm),
        })
    return in_maps


def assemble(results, gates, bv, Wo, bo):
    gates = np.asarray(gates, np.float32)
    bv = np.asarray(bv, np.float32)
    Wo = np.asarray(Wo, np.float32)
    bo = np.asarray(bo, np.float32)
    A = np.empty((B, N_HEADS, T, T), np.float32)
    out = np.zeros((B, T, D_MODEL), np.float32)
    for c in range(8):
        b, g = c // 4, c % 4
        A[b, g * HPC:(g + 1) * HPC] = results[c]["A"]
        out[b] += results[c]["out"]
    for b in range(B):
        corr = bo.astype(np.float32).copy()
        for h in range(N_HEADS):
            corr = corr + gates[b, h] * (
                bv[h * 64:(h + 1) * 64] @ Wo[h * 64:(h + 1) * 64, :]
            )
        out[b] += corr[None, :]
    return out, A


def kernel(x, gates, Wq, bq, Wk, bk, Wv, bv, Wo, bo):
    nc = _get_nc()
    in_maps = make_in_maps(x, gates, Wq, bq, Wk, bk, Wv, bv, Wo, bo)
    res = run_bass_kernel_spmd(nc, in_maps, list(range(8)))
    return assemble(res.results, gates, bv, Wo, bo)


# revision 9
# speedup vs baseline: 14.0992x; 14.0992x over previous
"""Trainium2 Bass kernel for CustomMHA (B=2, T=2048, D=768, H=12, Dh=64).

Sharding: 8 cores = 2 batches x 4 head-groups (3 heads each).
Each core computes, for its (b, heads) shard:
  - Q^T, K^T (transposed layouts) and V (natural) projections
  - scores in BOTH layouts on PE (natural for softmax + A output,
    transposed for the A@V contraction)
  - no-max softmax: exp on ACT with fused row-sum (accum_out),
    A = gate/rowsum * E scaled on DVE, streamed to DRAM
  - z^T = V^T E^T accumulated in PSUM, scaled by gate/rowsum, then
    out_partial = z @ Wo_rows on PE.
Host: gathers A shards, sums the 4 per-batch out partials, adds
bo and the (exact) bv correction  sum_h gate[b,h] * (bv_h @ Wo_h).
"""

import sys

if "/opt/trn_rl_repo" not in sys.path:
    sys.path.insert(0, "/opt/trn_rl_repo")

import math

import numpy as np

import concourse.bass as bass
import concourse.tile as tile
import concourse.mybir as mybir
from concourse import bacc
from concourse.bass_utils import run_bass_kernel_spmd
from concourse.masks import make_identity

F32 = mybir.dt.float32
F32R = mybir.dt.float32r
AF = mybir.ActivationFunctionType


def _r(ap):
    return ap

D_MODEL = 768
N_HEADS = 12
D_HEAD = 64
B, T = 2, 2048
HPC = 3          # heads per core
NB = T // 128    # 16 blocks of 128
KC = D_MODEL // 128  # 6 contraction chunks


def build_program():
    nc = bacc.Bacc("TRN2", target_bir_lowering=False, debug=False, num_devices=8)

    xb = nc.dram_tensor("x", [T, D_MODEL], F32, kind="ExternalInput")
    wq = nc.dram_tensor("wq", [D_MODEL, HPC * D_HEAD], F32, kind="ExternalInput")
    wk = nc.dram_tensor("wk", [D_MODEL, HPC * D_HEAD], F32, kind="ExternalInput")
    wv = nc.dram_tensor("wv", [D_MODEL, HPC * D_HEAD], F32, kind="ExternalInput")
    wo = nc.dram_tensor("wo", [HPC * D_HEAD, D_MODEL], F32, kind="ExternalInput")
    bq = nc.dram_tensor("bq", [HPC * D_HEAD, 1], F32, kind="ExternalInput")
    bk = nc.dram_tensor("bk", [HPC * D_HEAD, 1], F32, kind="ExternalInput")
    gmat = nc.dram_tensor("gmat", [128, HPC], F32, kind="ExternalInput")

    A_out = nc.dram_tensor("A", [HPC, T, T], F32, kind="ExternalOutput")
    o_out = nc.dram_tensor("out", [T, D_MODEL], F32, kind="ExternalOutput")

    c_dram = nc.dram_tensor("cscratch", [1, NB * 128], F32)

    from contextlib import ExitStack
    stack = ExitStack()
    with tile.TileContext(nc) as tc:
        consts = stack.enter_context(tc.tile_pool(name="consts", bufs=1))
        ident = consts.tile([128, 128], F32)
        make_identity(nc, ident[:])
        wq_t = consts.tile([128, KC, HPC * D_HEAD], F32R)
        wk_t = consts.tile([128, KC, HPC * D_HEAD], F32R)
        wv_t = consts.tile([128, KC, HPC * D_HEAD], F32R)
        nc.sync.dma_start(out=wq_t[:], in_=wq.rearrange("(k p) m -> p k m", p=128).bitcast(F32R))
        nc.sync.dma_start(out=wk_t[:], in_=wk.rearrange("(k p) m -> p k m", p=128).bitcast(F32R))
        nc.sync.dma_start(out=wv_t[:], in_=wv.rearrange("(k p) m -> p k m", p=128).bitcast(F32R))
        wo_t = consts.tile([128, 2, D_MODEL], F32R)
        nc.sync.dma_start(out=wo_t[:, 0, :], in_=wo[0:128, :].bitcast(F32R))
        nc.sync.dma_start(out=wo_t[0:64, 1, :], in_=wo[128:192, :].bitcast(F32R))
        bq_sb = consts.tile([128, 2], F32)
        bk_sb = consts.tile([128, 2], F32)
        nc.sync.dma_start(out=bq_sb[:, 0:1], in_=bq[0:128, :])
        nc.sync.dma_start(out=bq_sb[0:64, 1:2], in_=bq[128:192, :])
        nc.sync.dma_start(out=bk_sb[:, 0:1], in_=bk[0:128, :])
        nc.sync.dma_start(out=bk_sb[0:64, 1:2], in_=bk[128:192, :])
        gm_sb = consts.tile([128, HPC], F32)
        nc.sync.dma_start(out=gm_sb[:], in_=gmat[:])
        ones1 = consts.tile([1, 64], F32)
        nc.gpsimd.memset(ones1[:], 1.0)

        qkv = stack.enter_context(tc.tile_pool(name="qkv", bufs=1))
        qT01 = qkv.tile([128, T], F32R)
        qT2 = qkv.tile([64, T], F32R)
        kT01 = qkv.tile([128, T], F32R)
        kT2 = qkv.tile([64, T], F32R)
        V = qkv.tile([128, NB, HPC * D_HEAD], F32R)

        # ---- phase 1: load x, build x^T via PE transposes; also V ----
        with tc.tile_pool(name="xpools", bufs=1) as xpool, \
             tc.tile_pool(name="xTpool", bufs=1) as xTpool:
            x_all = xpool.tile([128, NB, D_MODEL], F32)
            nc.sync.dma_start(
                out=x_all[:], in_=xb.rearrange("(i p) d -> p i d", p=128)
            )
            xT = xTpool.tile([128, KC, T], F32R)
            with tc.tile_pool(name="psum_tr", bufs=4, space="PSUM") as ptr, \
                 tc.tile_pool(name="psum_v", bufs=2, space="PSUM") as pv:
                for i in range(NB):
                    for k in range(KC):
                        pt = ptr.tile([128, 128], F32)
                        nc.tensor.transpose(
                            pt[:], x_all[:, i, k * 128:(k + 1) * 128], ident[:]
                        )
                        nc.vector.tensor_copy(
                            xT[:, k, i * 128:(i + 1) * 128], pt[:]
                        )
                # V natural: [t, dh'] via lhsT = xT chunk, rhs = wv chunk
                for i in range(NB):
                    pvt = pv.tile([128, HPC * D_HEAD], F32)
                    for k in range(KC):
                        nc.tensor.matmul(
                            pvt[:],
                            xT[:, k, i * 128:(i + 1) * 128],
                            wv_t[:, k, :],
                            start=(k == 0),
                            stop=(k == KC - 1),
                        )
                    nc.vector.tensor_copy(V[:, i, :], pvt[:])

            # ---- phase 2: Q^T / K^T (stacked head pairs) ----
            with tc.tile_pool(name="psum_qkv", bufs=2, space="PSUM") as pq:
                for (dst, wt, bsb, lo, hi, col) in (
                    (qT01, wq_t, bq_sb, 0, 128, 0),
                    (qT2, wq_t, bq_sb, 128, 192, 1),
                    (kT01, wk_t, bk_sb, 0, 128, 0),
                    (kT2, wk_t, bk_sb, 128, 192, 1),
                ):
                    m = hi - lo
                    pt = pq.tile([128, T], F32, tag="pqkv")
                    for k in range(KC):
                        for n in range(4):
                            nc.tensor.matmul(
                                pt[0:m, n * 512:(n + 1) * 512],
                                _r(wt[:, k, lo:hi]),
                                _r(xT[:, k, n * 512:(n + 1) * 512]),
                                start=(k == 0),
                                stop=(k == KC - 1),
                            )
                    nc.vector.tensor_scalar(
                        out=dst[:], in0=pt[0:m, :],
                        scalar1=bsb[0:m, col:col + 1], scalar2=None,
                        op0=mybir.AluOpType.add,
                    )

        # ---- per-head attention ----
        epool = stack.enter_context(tc.tile_pool(name="epool", bufs=3))
        etpool = stack.enter_context(tc.tile_pool(name="etpool", bufs=3))
        cbpool = stack.enter_context(tc.tile_pool(name="cbpool", bufs=2))
        zpool = stack.enter_context(tc.tile_pool(name="zpool", bufs=1))
        small = stack.enter_context(tc.tile_pool(name="small", bufs=2))
        zTs_all = zpool.tile([128, T], F32R)
        zTs2 = zpool.tile([64, T], F32R)

        for h in range(HPC):
            if h < 2:
                qT_h = qT01[h * 64:(h + 1) * 64, :]
                kT_h = kT01[h * 64:(h + 1) * 64, :]
            else:
                qT_h = qT2[0:64, :]
                kT_h = kT2[0:64, :]

            r_all = small.tile([128, NB], F32, tag="r_all")
            c_all = small.tile([128, NB], F32, tag="c_all")
            tmp_r = small.tile([128, NB], F32, tag="tmp_r")

            # natural phase: softmax + A output
            with tc.tile_pool(name="psum_S", bufs=2, space="PSUM") as ps:
                for i in range(NB):
                    pt = ps.tile([128, T], F32, tag="S")
                    for n in range(4):
                        nc.tensor.matmul(
                            pt[:, n * 512:(n + 1) * 512],
                            _r(qT_h[:, i * 128:(i + 1) * 128]),
                            _r(kT_h[:, n * 512:(n + 1) * 512]),
                            start=True,
                            stop=True,
                        )
                    E = epool.tile([128, T], F32, tag="E")
                    nc.scalar.activation(
                        out=E[:], in_=pt[:], func=AF.Exp,
                        accum_out=r_all[:, i:i + 1],
                    )
                    nc.vector.reciprocal(
                        out=tmp_r[:, i:i + 1], in_=r_all[:, i:i + 1]
                    )
                    nc.vector.tensor_tensor(
                        out=c_all[:, i:i + 1],
                        in0=tmp_r[:, i:i + 1],
                        in1=gm_sb[:, h:h + 1],
                        op=mybir.AluOpType.mult,
                    )
                    nc.vector.tensor_scalar(
                        out=E[:], in0=E[:],
                        scalar1=c_all[:, i:i + 1], scalar2=None,
                        op0=mybir.AluOpType.mult,
                    )
                    nc.sync.dma_start(
                        out=A_out[h, i * 128:(i + 1) * 128, :], in_=E[:]
                    )

            # build C_bcast [64, T]: c as a free-dim row, replicated
            C_b = cbpool.tile([64, T], F32, tag="cb")
            with tc.tile_pool(name="psum_cb", bufs=1, space="PSUM") as pcb, \
                 tc.tile_pool(name="psum_ct", bufs=1, space="PSUM") as pct:
                ptc = pct.tile([NB, 128], F32)
                nc.tensor.transpose(ptc[:], c_all[:], ident[:])
                cT_sb = small.tile([NB, 128], F32, tag="cT")
                nc.vector.tensor_copy(cT_sb[:], ptc[:])
                nc.sync.dma_start(
                    out=c_dram.rearrange("a (i p) -> (a i) p", i=NB),
                    in_=cT_sb[:],
                )
                c_free = small.tile([1, T], F32, tag="cfree")
                nc.sync.dma_start(out=c_free[:], in_=c_dram[:])
                pcbt = pcb.tile([64, T], F32)
                for n in range(4):
                    nc.tensor.matmul(
                        pcbt[:, n * 512:(n + 1) * 512],
                        _r(ones1[:]),
                        _r(c_free[:, n * 512:(n + 1) * 512]),
                        start=True,
                        stop=True,
                    )
                nc.vector.tensor_copy(C_b[:], pcbt[:])

            # transposed phase: S^T -> exp -> z^T accumulation
            with tc.tile_pool(name="psum_ST", bufs=2, space="PSUM") as pst, \
                 tc.tile_pool(name="psum_zT", bufs=1, space="PSUM") as pzt:
                zT = pzt.tile([64, T], F32)
                for j in range(NB):
                    for half in range(2):
                        pt = pst.tile([128, 1024], F32, tag="ST")
                        for k2 in range(2):
                            nc.tensor.matmul(
                                pt[:, k2 * 512:(k2 + 1) * 512],
                                _r(kT_h[:, j * 128:(j + 1) * 128]),
                                _r(qT_h[:, half * 1024 + k2 * 512:
                                        half * 1024 + (k2 + 1) * 512]),
                                start=True,
                                stop=True,
                            )
                        ET = etpool.tile([128, 1024], F32R, tag="ET")
                        nc.scalar.activation(out=ET[:], in_=pt[:], func=AF.Exp)
                        for k2 in range(2):
                            nc.tensor.matmul(
                                zT[:, half * 1024 + k2 * 512:
                                   half * 1024 + (k2 + 1) * 512],
                                _r(V[:, j, h * 64:(h + 1) * 64]),
                                _r(ET[:, k2 * 512:(k2 + 1) * 512]),
                                start=(j == 0),
                                stop=(j == NB - 1),
                                skip_group_check=True,
                            )
                # scale columns by c (gate/rowsum) and park in SBUF
                if h < 2:
                    zdst = zTs_all[h * 64:(h + 1) * 64, :]
                else:
                    zdst = zTs2[0:64, :]
                nc.vector.tensor_tensor(
                    out=zdst, in0=zT[:], in1=C_b[:], op=mybir.AluOpType.mult
                )

        # ---- output projection ----
        with tc.tile_pool(name="psum_out", bufs=2, space="PSUM") as po, \
             tc.tile_pool(name="obuf", bufs=3) as obuf:
            for i2 in range(NB // 2):
                ot = obuf.tile([128, 2, D_MODEL], F32, tag="ot")
                for ii in range(2):
                    i = i2 * 2 + ii
                    pt = po.tile([128, D_MODEL], F32, tag="po")
                    for n, (n0, n1) in enumerate(((0, 512), (512, 768))):
                        nc.tensor.matmul(
                            pt[:, n0:n1],
                            _r(zTs_all[:, i * 128:(i + 1) * 128]),
                            _r(wo_t[:, 0, n0:n1]),
                            start=True,
                            stop=False,
                        )
                        nc.tensor.matmul(
                            pt[:, n0:n1],
                            _r(zTs2[:, i * 128:(i + 1) * 128]),
                            _r(wo_t[0:64, 1, n0:n1]),
                            start=False,
                            stop=True,
                        )
                    nc.vector.tensor_copy(ot[:, ii, :], pt[:])
                nc.sync.dma_start(
                    out=o_out[i2 * 256:(i2 + 1) * 256, :].rearrange(
                        "(g p) d -> p g d", p=128
                    ),
                    in_=ot[:],
                )

        stack.close()

    nc.compile()
    return nc


_NC = None


def _get_nc():
    global _NC
    if _NC is None:
        _NC = build_program()
    return _NC


def make_in_maps(x, gates, Wq, bq, Wk, bk, Wv, bv, Wo, bo):
    x = np.asarray(x, np.float32)
    gates = np.asarray(gates, np.float32)
    Wq, bq = np.asarray(Wq, np.float32), np.asarray(bq, np.float32)
    Wk, bk = np.asarray(Wk, np.float32), np.asarray(bk, np.float32)
    Wv = np.asarray(Wv, np.float32)
    Wo = np.asarray(Wo, np.float32)
    in_maps = []
    for c in range(8):
        b, g = c // 4, c % 4
        sl = slice(g * HPC * D_HEAD, (g + 1) * HPC * D_HEAD)
        gm = np.repeat(gates[b, g * HPC:(g + 1) * HPC][None, :], 128, axis=0)
        in_maps.append({
            "x": np.ascontiguousarray(x[b]),
            "wq": np.ascontiguousarray(Wq[:, sl]) / 8.0,
            "wk": np.ascontiguousarray(Wk[:, sl]),
            "wv": np.ascontiguousarray(Wv[:, sl]),
            "wo": np.ascontiguousarray(Wo[sl, :]),
            "bq": np.ascontiguousarray(bq[sl, None]) / 8.0,
            "bk": np.ascontiguousarray(bk[sl, None]),
            "gmat": np.ascontiguousarray(gm),
        })
    return in_maps


def assemble(results, gates, bv, Wo, bo):
    gates = np.asarray(gates, np.float32)
    bv = np.asarray(bv, np.float32)
    Wo = np.asarray(Wo, np.float32)
    bo = np.asarray(bo, np.float32)
    A = np.empty((B, N_HEADS, T, T), np.float32)
    out = np.zeros((B, T, D_MODEL), np.float32)
    for c in range(8):
        b, g = c // 4, c % 4
        A[b, g * HPC:(g + 1) * HPC] = results[c]["A"]
        out[b] += results[c]["out"]
    for b in range(B):
        corr = bo.astype(np.float32).copy()
        for h in range(N_HEADS):
            corr = corr + gates[b, h] * (
                bv[h * 64:(h + 1) * 64] @ Wo[h * 64:(h + 1) * 64, :]
            )
        out[b] += corr[None, :]
    return out, A


def kernel(x, gates, Wq, bq, Wk, bk, Wv, bv, Wo, bo):
    nc = _get_nc()
    in_maps = make_in_maps(x, gates, Wq, bq, Wk, bk, Wv, bv, Wo, bo)
    res = run_bass_kernel_spmd(nc, in_maps, list(range(8)))
    return assemble(res.results, gates, bv, Wo, bo)


# revision 14
# speedup vs baseline: 14.3949x; 1.0210x over previous
"""Trainium2 Bass kernel for CustomMHA (B=2, T=2048, D=768, H=12, Dh=64).

Sharding: 8 cores = 2 batches x 4 head-groups (3 heads each).
Each core computes, for its (b, heads) shard:
  - Q^T, K^T (transposed layouts) and V (natural) projections
  - scores in BOTH layouts on PE (natural for softmax + A output,
    transposed for the A@V contraction)
  - no-max softmax: exp on ACT with fused row-sum (accum_out),
    A = gate/rowsum * E scaled on DVE, streamed to DRAM
  - z^T = V^T E^T accumulated in PSUM, scaled by gate/rowsum, then
    out_partial = z @ Wo_rows on PE.
Host: gathers A shards, sums the 4 per-batch out partials, adds
bo and the (exact) bv correction  sum_h gate[b,h] * (bv_h @ Wo_h).
"""

import sys

if "/opt/trn_rl_repo" not in sys.path:
    sys.path.insert(0, "/opt/trn_rl_repo")

import math

import numpy as np

import concourse.bass as bass
import concourse.tile as tile
import concourse.mybir as mybir
from concourse import bacc
from concourse.bass_utils import run_bass_kernel_spmd
from concourse.masks import make_identity

F32 = mybir.dt.float32
F32R = mybir.dt.float32r
AF = mybir.ActivationFunctionType


def _r(ap):
    return ap

D_MODEL = 768
N_HEADS = 12
D_HEAD = 64
B, T = 2, 2048
HPC = 3          # heads per core
NB = T // 128    # 16 blocks of 128
KC = D_MODEL // 128  # 6 contraction chunks


def build_program():
    nc = bacc.Bacc("TRN2", target_bir_lowering=False, debug=False, num_devices=8)

    xb = nc.dram_tensor("x", [T, D_MODEL], F32, kind="ExternalInput")
    wq = nc.dram_tensor("wq", [D_MODEL, HPC * D_HEAD], F32, kind="ExternalInput")
    wk = nc.dram_tensor("wk", [D_MODEL, HPC * D_HEAD], F32, kind="ExternalInput")
    wv = nc.dram_tensor("wv", [D_MODEL, HPC * D_HEAD], F32, kind="ExternalInput")
    wo = nc.dram_tensor("wo", [HPC * D_HEAD, D_MODEL], F32, kind="ExternalInput")
    bq = nc.dram_tensor("bq", [HPC * D_HEAD, 1], F32, kind="ExternalInput")
    bk = nc.dram_tensor("bk", [HPC * D_HEAD, 1], F32, kind="ExternalInput")
    gmat = nc.dram_tensor("gmat", [128, HPC], F32, kind="ExternalInput")

    A_out = nc.dram_tensor("A", [HPC, T, T], F32, kind="ExternalOutput")
    o_out = nc.dram_tensor("out", [T, D_MODEL], F32, kind="ExternalOutput")

    c_dram = nc.dram_tensor("cscratch", [1, NB * 128], F32)

    from contextlib import ExitStack
    stack = ExitStack()
    with tile.TileContext(nc) as tc:
        consts = stack.enter_context(tc.tile_pool(name="consts", bufs=1))
        ident = consts.tile([128, 128], F32)
        make_identity(nc, ident[:])
        wq_t = consts.tile([128, KC, HPC * D_HEAD], F32R)
        wk_t = consts.tile([128, KC, HPC * D_HEAD], F32R)
        wv_t = consts.tile([128, KC, HPC * D_HEAD], F32R)
        nc.sync.dma_start(out=wq_t[:], in_=wq.rearrange("(k p) m -> p k m", p=128).bitcast(F32R))
        nc.sync.dma_start(out=wk_t[:], in_=wk.rearrange("(k p) m -> p k m", p=128).bitcast(F32R))
        nc.sync.dma_start(out=wv_t[:], in_=wv.rearrange("(k p) m -> p k m", p=128).bitcast(F32R))
        wo_t = consts.tile([128, 2, D_MODEL], F32R)
        nc.sync.dma_start(out=wo_t[:, 0, :], in_=wo[0:128, :].bitcast(F32R))
        nc.sync.dma_start(out=wo_t[0:64, 1, :], in_=wo[128:192, :].bitcast(F32R))
        bq_sb = consts.tile([128, 2], F32)
        bk_sb = consts.tile([128, 2], F32)
        nc.sync.dma_start(out=bq_sb[:, 0:1], in_=bq[0:128, :])
        nc.sync.dma_start(out=bq_sb[0:64, 1:2], in_=bq[128:192, :])
        nc.sync.dma_start(out=bk_sb[:, 0:1], in_=bk[0:128, :])
        nc.sync.dma_start(out=bk_sb[0:64, 1:2], in_=bk[128:192, :])
        gm_sb = consts.tile([128, HPC], F32)
        nc.sync.dma_start(out=gm_sb[:], in_=gmat[:])
        ones1 = consts.tile([1, 64], F32)
        nc.gpsimd.memset(ones1[:], 1.0)

        qkv = stack.enter_context(tc.tile_pool(name="qkv", bufs=1))
        qT01 = qkv.tile([128, T], F32R)
        qT2 = qkv.tile([64, T], F32R)
        kT01 = qkv.tile([128, T], F32R)
        kT2 = qkv.tile([64, T], F32R)
        V = qkv.tile([128, NB, HPC * D_HEAD], F32R)

        # ---- phase 1: load x, build x^T via PE transposes; also V ----
        with tc.tile_pool(name="xpools", bufs=1) as xpool, \
             tc.tile_pool(name="xTpool", bufs=1) as xTpool:
            x_all = xpool.tile([128, NB, D_MODEL], F32)
            for i4 in range(4):
                nc.sync.dma_start(
                    out=x_all[:, i4 * 4:(i4 + 1) * 4, :],
                    in_=xb[i4 * 512:(i4 + 1) * 512, :].rearrange(
                        "(i p) d -> p i d", p=128
                    ),
                )
            xT = xTpool.tile([128, KC, T], F32R)
            with tc.tile_pool(name="psum_tr", bufs=4, space="PSUM") as ptr:
                for i in range(NB):
                    for k in range(KC):
                        pt = ptr.tile([128, 128], F32)
                        nc.tensor.transpose(
                            pt[:], x_all[:, i, k * 128:(k + 1) * 128], ident[:]
                        )
                        nc.vector.tensor_copy(
                            xT[:, k, i * 128:(i + 1) * 128], pt[:]
                        )
            # ---- phase 2: Q^T / K^T (stacked head pairs), then V ----
            with tc.tile_pool(name="psum_qkv", bufs=1, space="PSUM") as pq, \
                 tc.tile_pool(name="psum_v2", bufs=2, space="PSUM") as pv2:
                for (dst, wt, bsb, lo, hi, col) in (
                    (qT01, wq_t, bq_sb, 0, 128, 0),
                    (qT2, wq_t, bq_sb, 128, 192, 1),
                    (kT01, wk_t, bk_sb, 0, 128, 0),
                    (kT2, wk_t, bk_sb, 128, 192, 1),
                ):
                    m = hi - lo
                    pt = pq.tile([128, T], F32, tag="pqkv")
                    for k in range(KC):
                        for n in range(4):
                            nc.tensor.matmul(
                                pt[0:m, n * 512:(n + 1) * 512],
                                _r(wt[:, k, lo:hi]),
                                _r(xT[:, k, n * 512:(n + 1) * 512]),
                                start=(k == 0),
                                stop=(k == KC - 1),
                            )
                    nc.vector.tensor_scalar(
                        out=dst[:], in0=pt[0:m, :],
                        scalar1=bsb[0:m, col:col + 1], scalar2=None,
                        op0=mybir.AluOpType.add,
                    )
                # V natural: [t, dh'] via lhsT = xT chunk, rhs = wv chunk
                for i in range(NB):
                    pvt = pv2.tile([128, HPC * D_HEAD], F32)
                    for k in range(KC):
                        nc.tensor.matmul(
                            pvt[:],
                            xT[:, k, i * 128:(i + 1) * 128],
                            wv_t[:, k, :],
                            start=(k == 0),
                            stop=(k == KC - 1),
                        )
                    nc.vector.tensor_copy(V[:, i, :], pvt[:])

        # ---- per-head attention ----
        epool = stack.enter_context(tc.tile_pool(name="epool", bufs=3))
        etpool = stack.enter_context(tc.tile_pool(name="etpool", bufs=4))
        cbpool = stack.enter_context(tc.tile_pool(name="cbpool", bufs=2))
        zpool = stack.enter_context(tc.tile_pool(name="zpool", bufs=1))
        small = stack.enter_context(tc.tile_pool(name="small", bufs=2))
        zTs_all = zpool.tile([128, T], F32R)
        zTs2 = zpool.tile([64, T], F32R)

        for h in range(HPC):
            if h < 2:
                qT_h = qT01[h * 64:(h + 1) * 64, :]
                kT_h = kT01[h * 64:(h + 1) * 64, :]
            else:
                qT_h = qT2[0:64, :]
                kT_h = kT2[0:64, :]

            r_all = small.tile([128, NB], F32, tag="r_all")
            c_all = small.tile([128, NB], F32, tag="c_all")
            tmp_r = small.tile([128, NB], F32, tag="tmp_r")

            # natural phase: softmax + A output
            with tc.tile_pool(name="psum_S", bufs=2, space="PSUM") as ps:
                for i2 in range(NB // 2):
                    E = epool.tile([128, 2, T], F32, tag="E")
                    for g in range(2):
                        i = i2 * 2 + g
                        pt = ps.tile([128, T], F32, tag="S")
                        for n in range(4):
                            nc.tensor.matmul(
                                pt[:, n * 512:(n + 1) * 512],
                                _r(qT_h[:, i * 128:(i + 1) * 128]),
                                _r(kT_h[:, n * 512:(n + 1) * 512]),
                                start=True,
                                stop=True,
                            )
                        nc.scalar.activation(
                            out=E[:, g, :], in_=pt[:], func=AF.Exp,
                            accum_out=r_all[:, i:i + 1],
                        )
                        nc.vector.reciprocal(
                            out=tmp_r[:, i:i + 1], in_=r_all[:, i:i + 1]
                        )
                        nc.vector.tensor_tensor(
                            out=c_all[:, i:i + 1],
                            in0=tmp_r[:, i:i + 1],
                            in1=gm_sb[:, h:h + 1],
                            op=mybir.AluOpType.mult,
                        )
                        nc.vector.tensor_scalar(
                            out=E[:, g, :], in0=E[:, g, :],
                            scalar1=c_all[:, i:i + 1], scalar2=None,
                            op0=mybir.AluOpType.mult,
                        )
                    nc.scalar.dma_start(
                        out=A_out[h, i2 * 256:(i2 + 1) * 256, :].rearrange(
                            "(g p) s -> p g s", p=128
                        ),
                        in_=E[:],
                    )

            # build C_bcast [64, T]: c as a free-dim row, replicated
            C_b = cbpool.tile([64, T], F32, tag="cb")
            with tc.tile_pool(name="psum_cb", bufs=1, space="PSUM") as pcb, \
                 tc.tile_pool(name="psum_ct", bufs=1, space="PSUM") as pct:
                ptc = pct.tile([NB, 128], F32)
                nc.tensor.transpose(ptc[:], c_all[:], ident[:])
                cT_sb = small.tile([NB, 128], F32, tag="cT")
                nc.vector.tensor_copy(cT_sb[:], ptc[:])
                nc.sync.dma_start(
                    out=c_dram.rearrange("a (i p) -> (a i) p", i=NB),
                    in_=cT_sb[:],
                )
                c_free = small.tile([1, T], F32, tag="cfree")
                nc.sync.dma_start(out=c_free[:], in_=c_dram[:])
                pcbt = pcb.tile([64, T], F32)
                for n in range(4):
                    nc.tensor.matmul(
                        pcbt[:, n * 512:(n + 1) * 512],
                        _r(ones1[:]),
                        _r(c_free[:, n * 512:(n + 1) * 512]),
                        start=True,
                        stop=True,
                    )
                nc.vector.tensor_copy(C_b[:], pcbt[:])

            # transposed phase: S^T -> exp -> z^T accumulation
            with tc.tile_pool(name="psum_ST", bufs=2, space="PSUM") as pst, \
                 tc.tile_pool(name="psum_zT", bufs=1, space="PSUM") as pzt:
                zT = pzt.tile([64, T], F32)
                for j in range(NB):
                    for half in range(2):
                        pt = pst.tile([128, 1024], F32, tag="ST")
                        for k2 in range(2):
                            nc.tensor.matmul(
                                pt[:, k2 * 512:(k2 + 1) * 512],
                                _r(kT_h[:, j * 128:(j + 1) * 128]),
                                _r(qT_h[:, half * 1024 + k2 * 512:
                                        half * 1024 + (k2 + 1) * 512]),
                                start=True,
                                stop=True,
                            )
                        ET = etpool.tile([128, 1024], F32R, tag="ET")
                        nc.scalar.activation(out=ET[:], in_=pt[:], func=AF.Exp)
                        for k2 in range(2):
                            nc.tensor.matmul(
                                zT[:, half * 1024 + k2 * 512:
                                   half * 1024 + (k2 + 1) * 512],
                                _r(V[:, j, h * 64:(h + 1) * 64]),
                                _r(ET[:, k2 * 512:(k2 + 1) * 512]),
                                start=(j == 0),
                                stop=(j == NB - 1),
                                skip_group_check=True,
                            )
                # scale columns by c (gate/rowsum) and park in SBUF
                if h < 2:
                    zdst = zTs_all[h * 64:(h + 1) * 64, :]
                else:
                    zdst = zTs2[0:64, :]
                nc.vector.tensor_tensor(
                    out=zdst, in0=zT[:], in1=C_b[:], op=mybir.AluOpType.mult
                )

        # ---- output projection ----
        with tc.tile_pool(name="psum_out", bufs=2, space="PSUM") as po, \
             tc.tile_pool(name="obuf", bufs=3) as obuf:
            for i2 in range(NB // 2):
                ot = obuf.tile([128, 2, D_MODEL], F32, tag="ot")
                for ii in range(2):
                    i = i2 * 2 + ii
                    pt = po.tile([128, D_MODEL], F32, tag="po")
                    for n, (n0, n1) in enumerate(((0, 512), (512, 768))):
                        nc.tensor.matmul(
                            pt[:, n0:n1],
                            _r(zTs_all[:, i * 128:(i + 1) * 128]),
                            _r(wo_t[:, 0, n0:n1]),
                            start=True,
                            stop=False,
                        )
                        nc.tensor.matmul(
                            pt[:, n0:n1],
                            _r(zTs2[:, i * 128:(i + 1) * 128]),
                            _r(wo_t[0:64, 1, n0:n1]),
                            start=False,
                            stop=True,
                        )
                    nc.vector.tensor_copy(ot[:, ii, :], pt[:])
                nc.scalar.dma_start(
                    out=o_out[i2 * 256:(i2 + 1) * 256, :].rearrange(
                        "(g p) d -> p g d", p=128
                    ),
                    in_=ot[:],
                )

        stack.close()

    nc.compile()
    return nc


_NC = None


def _get_nc():
    global _NC
    if _NC is None:
        _NC = build_program()
    return _NC


def make_in_maps(x, gates, Wq, bq, Wk, bk, Wv, bv, Wo, bo):
    x = np.asarray(x, np.float32)
    gates = np.asarray(gates, np.float32)
    Wq, bq = np.asarray(Wq, np.float32), np.asarray(bq, np.float32)
    Wk, bk = np.asarray(Wk, np.float32), np.asarray(bk, np.float32)
    Wv = np.asarray(Wv, np.float32)
    Wo = np.asarray(Wo, np.float32)
    in_maps = []
    for c in range(8):
        b, g = c // 4, c % 4
        sl = slice(g * HPC * D_HEAD, (g + 1) * HPC * D_HEAD)
        gm = np.repeat(gates[b, g * HPC:(g + 1) * HPC][None, :], 128, axis=0)
        in_maps.append({
            "x": np.ascontiguousarray(x[b]),
            "wq": np.ascontiguousarray(Wq[:, sl]) / 8.0,
            "wk": np.ascontiguousarray(Wk[:, sl]),
            "wv": np.ascontiguousarray(Wv[:, sl]),
            "wo": np.ascontiguousarray(Wo[sl, :]),
            "bq": np.ascontiguousarray(bq[sl, None]) / 8.0,
            "bk": np.ascontiguousarray(bk[sl, None]),
            "gmat": np.ascontiguousarray(gm),
        })
    return in_maps


def assemble(results, gates, bv, Wo, bo):
    gates = np.asarray(gates, np.float32)
    bv = np.asarray(bv, np.float32)
    Wo = np.asarray(Wo, np.float32)
    bo = np.asarray(bo, np.float32)
    A = np.empty((B, N_HEADS, T, T), np.float32)
    out = np.zeros((B, T, D_MODEL), np.float32)
    for c in range(8):
        b, g = c // 4, c % 4
        A[b, g * HPC:(g + 1) * HPC] = results[c]["A"]
        out[b] += results[c]["out"]
    for b in range(B):
        corr = bo.astype(np.float32).copy()
        for h in range(N_HEADS):
            corr = corr + gates[b, h] * (
                bv[h * 64:(h + 1) * 64] @ Wo[h * 64:(h + 1) * 64, :]
            )
        out[b] += corr[None, :]
    return out, A


def kernel(x, gates, Wq, bq, Wk, bk, Wv, bv, Wo, bo):
    nc = _get_nc()
    in_maps = make_in_maps(x, gates, Wq, bq, Wk, bk, Wv, bv, Wo, bo)
    res = run_bass_kernel_spmd(nc, in_maps, list(range(8)))
    return assemble(res.results, gates, bv, Wo, bo)


# revision 17
# speedup vs baseline: 14.4926x; 1.0068x over previous
"""Trainium2 Bass kernel for CustomMHA (B=2, T=2048, D=768, H=12, Dh=64).

Sharding: 8 cores = 2 batches x 4 head-groups (3 heads each).
Each core computes, for its (b, heads) shard:
  - Q^T, K^T (transposed layouts) and V (natural) projections
  - scores in BOTH layouts on PE (natural for softmax + A output,
    transposed for the A@V contraction)
  - no-max softmax: exp on ACT with fused row-sum (accum_out),
    A = gate/rowsum * E scaled on DVE, streamed to DRAM
  - z^T = V^T E^T accumulated in PSUM, scaled by gate/rowsum, then
    out_partial = z @ Wo_rows on PE.
Host: gathers A shards, sums the 4 per-batch out partials, adds
bo and the (exact) bv correction  sum_h gate[b,h] * (bv_h @ Wo_h).
"""

import sys

if "/opt/trn_rl_repo" not in sys.path:
    sys.path.insert(0, "/opt/trn_rl_repo")

import math

import numpy as np

import concourse.bass as bass
import concourse.tile as tile
import concourse.mybir as mybir
from concourse import bacc
from concourse.bass_utils import run_bass_kernel_spmd
from concourse.masks import make_identity

F32 = mybir.dt.float32
F32R = mybir.dt.float32r
AF = mybir.ActivationFunctionType


def _r(ap):
    return ap

D_MODEL = 768
N_HEADS = 12
D_HEAD = 64
B, T = 2, 2048
HPC = 3          # heads per core
NB = T // 128    # 16 blocks of 128
KC = D_MODEL // 128  # 6 contraction chunks


def build_program():
    nc = bacc.Bacc("TRN2", target_bir_lowering=False, debug=False, num_devices=8)

    xb = nc.dram_tensor("x", [T, D_MODEL], F32, kind="ExternalInput")
    wq = nc.dram_tensor("wq", [D_MODEL, HPC * D_HEAD], F32, kind="ExternalInput")
    wk = nc.dram_tensor("wk", [D_MODEL, HPC * D_HEAD], F32, kind="ExternalInput")
    wv = nc.dram_tensor("wv", [D_MODEL, HPC * D_HEAD], F32, kind="ExternalInput")
    wo = nc.dram_tensor("wo", [HPC * D_HEAD, D_MODEL], F32, kind="ExternalInput")
    bq = nc.dram_tensor("bq", [HPC * D_HEAD, 1], F32, kind="ExternalInput")
    bk = nc.dram_tensor("bk", [HPC * D_HEAD, 1], F32, kind="ExternalInput")
    gmat = nc.dram_tensor("gmat", [128, HPC], F32, kind="ExternalInput")

    A_out = nc.dram_tensor("A", [HPC, T, T], F32, kind="ExternalOutput")
    o_out = nc.dram_tensor("out", [T, D_MODEL], F32, kind="ExternalOutput")

    c_dram = nc.dram_tensor("cscratch", [1, NB * 128], F32)

    from contextlib import ExitStack
    stack = ExitStack()
    with tile.TileContext(nc) as tc:
        consts = stack.enter_context(tc.tile_pool(name="consts", bufs=1))
        ident = consts.tile([128, 128], F32)
        make_identity(nc, ident[:])
        wo_t = consts.tile([128, 2, D_MODEL], F32R)
        nc.sync.dma_start(out=wo_t[:, 0, :], in_=wo[0:128, :].bitcast(F32R))
        nc.sync.dma_start(out=wo_t[0:64, 1, :], in_=wo[128:192, :].bitcast(F32R))
        bq_sb = consts.tile([128, 2], F32)
        bk_sb = consts.tile([128, 2], F32)
        nc.sync.dma_start(out=bq_sb[:, 0:1], in_=bq[0:128, :])
        nc.sync.dma_start(out=bq_sb[0:64, 1:2], in_=bq[128:192, :])
        nc.sync.dma_start(out=bk_sb[:, 0:1], in_=bk[0:128, :])
        nc.sync.dma_start(out=bk_sb[0:64, 1:2], in_=bk[128:192, :])
        gm_sb = consts.tile([128, HPC], F32)
        nc.sync.dma_start(out=gm_sb[:], in_=gmat[:])
        ones1 = consts.tile([1, 64], F32)
        nc.gpsimd.memset(ones1[:], 1.0)

        qkv = stack.enter_context(tc.tile_pool(name="qkv", bufs=1))
        qT01 = qkv.tile([128, T], F32R)
        qT2 = qkv.tile([64, T], F32R)
        kT01 = qkv.tile([128, T], F32R)
        kT2 = qkv.tile([64, T], F32R)
        V = qkv.tile([128, NB, HPC * D_HEAD], F32R)

        # ---- phase 1: load x, build x^T via PE transposes; also V ----
        with tc.tile_pool(name="xpools", bufs=1) as xpool, \
             tc.tile_pool(name="xTpool", bufs=1) as xTpool:
            x_all = xpool.tile([128, NB, D_MODEL], F32)
            for i4 in range(4):
                nc.sync.dma_start(
                    out=x_all[:, i4 * 4:(i4 + 1) * 4, :],
                    in_=xb[i4 * 512:(i4 + 1) * 512, :].rearrange(
                        "(i p) d -> p i d", p=128
                    ),
                )
            xT = xTpool.tile([128, KC, T], F32R)
            wq_t = xTpool.tile([128, KC, HPC * D_HEAD], F32R)
            wk_t = xTpool.tile([128, KC, HPC * D_HEAD], F32R)
            wv_t = xTpool.tile([128, KC, HPC * D_HEAD], F32R)
            nc.sync.dma_start(out=wq_t[:], in_=wq.rearrange("(k p) m -> p k m", p=128).bitcast(F32R))
            nc.sync.dma_start(out=wk_t[:], in_=wk.rearrange("(k p) m -> p k m", p=128).bitcast(F32R))
            nc.sync.dma_start(out=wv_t[:], in_=wv.rearrange("(k p) m -> p k m", p=128).bitcast(F32R))
            with tc.tile_pool(name="psum_tr", bufs=4, space="PSUM") as ptr:
                for i in range(NB):
                    for k in range(KC):
                        pt = ptr.tile([128, 128], F32)
                        nc.tensor.transpose(
                            pt[:], x_all[:, i, k * 128:(k + 1) * 128], ident[:]
                        )
                        nc.vector.tensor_copy(
                            xT[:, k, i * 128:(i + 1) * 128], pt[:]
                        )
            # ---- phase 2: Q^T / K^T (stacked head pairs), then V ----
            with tc.tile_pool(name="psum_qkv", bufs=1, space="PSUM") as pq, \
                 tc.tile_pool(name="psum_v2", bufs=2, space="PSUM") as pv2:
                for (dst, wt, bsb, lo, hi, col) in (
                    (qT01, wq_t, bq_sb, 0, 128, 0),
                    (qT2, wq_t, bq_sb, 128, 192, 1),
                    (kT01, wk_t, bk_sb, 0, 128, 0),
                    (kT2, wk_t, bk_sb, 128, 192, 1),
                ):
                    m = hi - lo
                    pt = pq.tile([128, T], F32, tag="pqkv")
                    for k in range(KC):
                        for n in range(4):
                            nc.tensor.matmul(
                                pt[0:m, n * 512:(n + 1) * 512],
                                _r(wt[:, k, lo:hi]),
                                _r(xT[:, k, n * 512:(n + 1) * 512]),
                                start=(k == 0),
                                stop=(k == KC - 1),
                            )
                    nc.vector.tensor_scalar(
                        out=dst[:], in0=pt[0:m, :],
                        scalar1=bsb[0:m, col:col + 1], scalar2=None,
                        op0=mybir.AluOpType.add,
                    )
                # V natural: [t, dh'] via lhsT = xT chunk, rhs = wv chunk
                for i in range(NB):
                    pvt = pv2.tile([128, HPC * D_HEAD], F32)
                    for k in range(KC):
                        nc.tensor.matmul(
                            pvt[:],
                            xT[:, k, i * 128:(i + 1) * 128],
                            wv_t[:, k, :],
                            start=(k == 0),
                            stop=(k == KC - 1),
                        )
                    nc.vector.tensor_copy(V[:, i, :], pvt[:])

        # ---- per-head attention ----
        epool = stack.enter_context(tc.tile_pool(name="epool", bufs=4))
        etpool = stack.enter_context(tc.tile_pool(name="etpool", bufs=4))
        cbpool = stack.enter_context(tc.tile_pool(name="cbpool", bufs=2))
        zpool = stack.enter_context(tc.tile_pool(name="zpool", bufs=1))
        small = stack.enter_context(tc.tile_pool(name="small", bufs=2))
        zTs_all = zpool.tile([128, T], F32R)
        zTs2 = zpool.tile([64, T], F32R)

        for h in range(HPC):
            if h < 2:
                qT_h = qT01[h * 64:(h + 1) * 64, :]
                kT_h = kT01[h * 64:(h + 1) * 64, :]
            else:
                qT_h = qT2[0:64, :]
                kT_h = kT2[0:64, :]

            r_all = small.tile([128, NB], F32, tag="r_all")
            c_all = small.tile([128, NB], F32, tag="c_all")
            tmp_r = small.tile([128, NB], F32, tag="tmp_r")

            # natural phase: softmax + A output
            with tc.tile_pool(name="psum_S", bufs=2, space="PSUM") as ps:
                for i2 in range(NB // 2):
                    E = epool.tile([128, 2, T], F32, tag="E")
                    for g in range(2):
                        i = i2 * 2 + g
                        pt = ps.tile([128, T], F32, tag="S")
                        for n in range(4):
                            nc.tensor.matmul(
                                pt[:, n * 512:(n + 1) * 512],
                                _r(qT_h[:, i * 128:(i + 1) * 128]),
                                _r(kT_h[:, n * 512:(n + 1) * 512]),
                                start=True,
                                stop=True,
                            )
                        nc.scalar.activation(
                            out=E[:, g, :], in_=pt[:], func=AF.Exp,
                            accum_out=r_all[:, i:i + 1],
                        )
                        nc.vector.reciprocal(
                            out=tmp_r[:, i:i + 1], in_=r_all[:, i:i + 1]
                        )
                        nc.vector.tensor_tensor(
                            out=c_all[:, i:i + 1],
                            in0=tmp_r[:, i:i + 1],
                            in1=gm_sb[:, h:h + 1],
                            op=mybir.AluOpType.mult,
                        )
                        nc.vector.tensor_scalar(
                            out=E[:, g, :], in0=E[:, g, :],
                            scalar1=c_all[:, i:i + 1], scalar2=None,
                            op0=mybir.AluOpType.mult,
                        )
                    nc.scalar.dma_start(
                        out=A_out[h, i2 * 256:(i2 + 1) * 256, :].rearrange(
                            "(g p) s -> p g s", p=128
                        ),
                        in_=E[:],
                    )

            # build C_bcast [64, T]: c as a free-dim row, replicated
            C_b = cbpool.tile([64, T], F32, tag="cb")
            with tc.tile_pool(name="psum_cb", bufs=1, space="PSUM") as pcb, \
                 tc.tile_pool(name="psum_ct", bufs=1, space="PSUM") as pct:
                ptc = pct.tile([NB, 128], F32)
                nc.tensor.transpose(ptc[:], c_all[:], ident[:])
                cT_sb = small.tile([NB, 128], F32, tag="cT")
                nc.vector.tensor_copy(cT_sb[:], ptc[:])
                nc.sync.dma_start(
                    out=c_dram.rearrange("a (i p) -> (a i) p", i=NB),
                    in_=cT_sb[:],
                )
                c_free = small.tile([1, T], F32, tag="cfree")
                nc.sync.dma_start(out=c_free[:], in_=c_dram[:])
                pcbt = pcb.tile([64, T], F32)
                for n in range(4):
                    nc.tensor.matmul(
                        pcbt[:, n * 512:(n + 1) * 512],
                        _r(ones1[:]),
                        _r(c_free[:, n * 512:(n + 1) * 512]),
                        start=True,
                        stop=True,
                    )
                nc.vector.tensor_copy(C_b[:], pcbt[:])

            # transposed phase: S^T -> exp -> z^T accumulation
            with tc.tile_pool(name="psum_ST", bufs=2, space="PSUM") as pst, \
                 tc.tile_pool(name="psum_zT", bufs=1, space="PSUM") as pzt:
                zT = pzt.tile([64, T], F32)
                for j in range(NB):
                    for half in range(2):
                        pt = pst.tile([128, 1024], F32, tag="ST")
                        for k2 in range(2):
                            nc.tensor.matmul(
                                pt[:, k2 * 512:(k2 + 1) * 512],
                                _r(kT_h[:, j * 128:(j + 1) * 128]),
                                _r(qT_h[:, half * 1024 + k2 * 512:
                                        half * 1024 + (k2 + 1) * 512]),
                                start=True,
                                stop=True,
                            )
                        ET = etpool.tile([128, 1024], F32R, tag="ET")
                        nc.scalar.activation(out=ET[:], in_=pt[:], func=AF.Exp)
                        for k2 in range(2):
                            nc.tensor.matmul(
                                zT[:, half * 1024 + k2 * 512:
                                   half * 1024 + (k2 + 1) * 512],
                                _r(V[:, j, h * 64:(h + 1) * 64]),
                                _r(ET[:, k2 * 512:(k2 + 1) * 512]),
                                start=(j == 0),
                                stop=(j == NB - 1),
                                skip_group_check=True,
                            )
                # scale columns by c (gate/rowsum) and park in SBUF
                if h < 2:
                    zdst = zTs_all[h * 64:(h + 1) * 64, :]
                else:
                    zdst = zTs2[0:64, :]
                nc.vector.tensor_tensor(
                    out=zdst, in0=zT[:], in1=C_b[:], op=mybir.AluOpType.mult
                )

        # ---- output projection ----
        with tc.tile_pool(name="psum_out", bufs=2, space="PSUM") as po, \
             tc.tile_pool(name="obuf", bufs=3) as obuf:
            for i2 in range(NB // 2):
                ot = obuf.tile([128, 2, D_MODEL], F32, tag="ot")
                for ii in range(2):
                    i = i2 * 2 + ii
                    pt = po.tile([128, D_MODEL], F32, tag="po")
                    for n, (n0, n1) in enumerate(((0, 512), (512, 768))):
                        nc.tensor.matmul(
                            pt[:, n0:n1],
                            _r(zTs_all[:, i * 128:(i + 1) * 128]),
                            _r(wo_t[:, 0, n0:n1]),
                            start=True,
                            stop=False,
                        )
                        nc.tensor.matmul(
                            pt[:, n0:n1],
                            _r(zTs2[:, i * 128:(i + 1) * 128]),
                            _r(wo_t[0:64, 1, n0:n1]),
                            start=False,
                            stop=True,
                        )
                    nc.vector.tensor_copy(ot[:, ii, :], pt[:])
                nc.scalar.dma_start(
                    out=o_out[i2 * 256:(i2 + 1) * 256, :].rearrange(
                        "(g p) d -> p g d", p=128
                    ),
                    in_=ot[:],
                )

        stack.close()

    nc.compile()
    return nc


_NC = None


def _get_nc():
    global _NC
    if _NC is None:
        _NC = build_program()
    return _NC


def make_in_maps(x, gates, Wq, bq, Wk, bk, Wv, bv, Wo, bo):
    x = np.asarray(x, np.float32)
    gates = np.asarray(gates, np.float32)
    Wq, bq = np.asarray(Wq, np.float32), np.asarray(bq, np.float32)
    Wk, bk = np.asarray(Wk, np.float32), np.asarray(bk, np.float32)
    Wv = np.asarray(Wv, np.float32)
    Wo = np.asarray(Wo, np.float32)
    in_maps = []
    for c in range(8):
        b, g = c // 4, c % 4
        sl = slice(g * HPC * D_HEAD, (g + 1) * HPC * D_HEAD)
        gm = np.repeat(gates[b, g * HPC:(g + 1) * HPC][None, :], 128, axis=0)
        in_maps.append({
            "x": np.ascontiguousarray(x[b]),
            "wq": np.ascontiguousarray(Wq[:, sl]) / 8.0,
            "wk": np.ascontiguousarray(Wk[:, sl]),
            "wv": np.ascontiguousarray(Wv[:, sl]),
            "wo": np.ascontiguousarray(Wo[sl, :]),
            "bq": np.ascontiguousarray(bq[sl, None]) / 8.0,
            "bk": np.ascontiguousarray(bk[sl, None]),
            "gmat": np.ascontiguousarray(gm),
        })
    return in_maps


def assemble(results, gates, bv, Wo, bo):
    gates = np.asarray(gates, np.float32)
    bv = np.asarray(bv, np.float32)
    Wo = np.asarray(Wo, np.float32)
    bo = np.asarray(bo, np.float32)
    A = np.empty((B, N_HEADS, T, T), np.float32)
    out = np.zeros((B, T, D_MODEL), np.float32)
    for c in range(8):
        b, g = c // 4, c % 4
        A[b, g * HPC:(g + 1) * HPC] = results[c]["A"]
        out[b] += results[c]["out"]
    for b in range(B):
        corr = bo.astype(np.float32).copy()
        for h in range(N_HEADS):
            corr = corr + gates[b, h] * (
                bv[h * 64:(h + 1) * 64] @ Wo[h * 64:(h + 1) * 64, :]
            )
        out[b] += corr[None, :]
    return out, A


def kernel(x, gates, Wq, bq, Wk, bk, Wv, bv, Wo, bo):
    nc = _get_nc()
    in_maps = make_in_maps(x, gates, Wq, bq, Wk, bk, Wv, bv, Wo, bo)
    res = run_bass_kernel_spmd(nc, in_maps, list(range(8)))
    return assemble(res.results, gates, bv, Wo, bo)
